# revision 16
# baseline (speedup 1.0000x reference)
"""Trainium2 Bass kernel for nn_BatchContrastLoss (InfoNCE-style contrastive loss).

Reference computation:
    sim[i,j]  = cos(que_i, ans_j)            (eps-guarded norms)
    logits    = sim / 0.07
    loss      = -mean_i(log_softmax(logits, axis=1)[i,i])

Sharding: data-parallel over rows of que across 8 NeuronCores. Each core
computes its [512, 4096] logits slab against the full ans batch and reduces
each row to a softmax denominator sum_j exp(logits[i,j]). The host takes
log + mean and subtracts the diagonal (the "all-reduce" of the hint).

Design (v5; baseline v1 was 101us, DVE/ScalarE-bound):
  - Row norms are folded into the fp8 quantization on the host: rows are
    normalized to unit length, scaled by 16 (keeps e4m3 mantissa well fed),
    and quantized. The device needs NO norm computation: psum =
    (16*qhat)·(16*ahat) = 256*cos and the exp drain folds 1/(256*gamma)
    into its free affine scale. The diagonal logits_ii are computed exactly
    on the host in f64 (O(B*D), negligible).
  - fp8e4m3 DoubleRow matmuls (K=256/instr, N=512): measured 216ns
    issue-to-issue warm => 128 MMs ~ 27.6us/core floor.
  - Loop order (g: 1024-col group, c: 512-col bank, t: k-pair, m: row tile)
    puts m INNERMOST so each arriving 128KB ans chunk feeds 4 matmuls
    (864ns) while the DMA pipe delivers the next chunk (~320-420ns): after
    the first chunk lands, the PE never waits for DMA again.
  - Drains are per [128,512] bank, split across two engines so neither is
    critical: ScalarE does in-place Exp (~720ns), idle VectorE does the
    row-sum reduction (~330ns). 32 of each, both well under the PE's 27.6us.
  - DMA: all on the SP HWDGE ring in consumption order. Each dma_start costs
    ~630ns of issue time on the Sync queue and the pipe has ~1.9us fill
    latency, so the front is fine-grained (4x128KB que tiles + 8x128KB group
    0 chunks) and the back is batched (6x512KB (g,c) blocks).
  - The PE clock gate (HAM) needs ~3.4us of sustained activity to unthrottle
    from 1.2 to 2.4 GHz; N_WARM dummy matmuls on a zeroed scratch tile keep
    the PE busy from block start until the first chunk lands. A dummy Exp
    pulls the one-time ~2.7us activation table load off the critical path.
"""

import numpy as np

import concourse.bass as bass
import concourse.mybir as mybir
import concourse.tile as tile
from concourse import bacc
from concourse.bass_utils import run_bass_kernel_spmd

# Problem constants (self-contained; the harness provides only the inputs).
B = 4096  # rows of que_batch / ans_batch
D = 1024  # feature dim
NCORES = 8
NB = B // NCORES  # local que rows per core = 512
P = 128  # SBUF partitions
KT2 = 4  # k-pair tiles (each DoubleRow matmul contracts 256 dims)
NW = 512  # matmul moving width = one fp32 PSUM bank
G = 4  # ans column groups of 1024
MT = NB // P  # 4 row tiles of 128
GAMA = 0.07
EPS = 1e-8
SCALE = 16.0  # host quantization scale on unit rows
EXP_SCALE = 1.0 / (SCALE * SCALE * GAMA)  # psum -> logits
N_WARM = 16  # dummy matmuls bridging block start -> first chunk arrival

F32 = mybir.dt.float32
FP8 = mybir.dt.float8e4  # e4m3
DR = mybir.MatmulPerfMode.DoubleRow
AF = mybir.ActivationFunctionType
AX = mybir.AxisListType
ALU = mybir.AluOpType

OUTPUT_NAMES = ["s_out"]


def _build_program():
    nc = bacc.Bacc(
        "TRN2", target_bir_lowering=False, debug=False, num_devices=NCORES
    )

    # qPK[m, p, 2t+i, mm] = q16hat_fp8[local row 128m+mm, d=256t+128i+p]
    qPK = nc.dram_tensor("qPK", [MT, P, 2 * KT2, P], FP8, kind="ExternalInput").ap()
    # cPK[4c+t, p, i, j] = a16hat_fp8[col 512c+j, d=256t+128i+p]   (group 0)
    cPK = nc.dram_tensor("cPK", [8, P, 2, NW], FP8, kind="ExternalInput").ap()
    # bPK[2(g-1)+c, p, 2t+i, j] = a16hat_fp8[col 1024g+512c+j, d=256t+128i+p]
    bPK = nc.dram_tensor("bPK", [6, P, 2 * KT2, NW], FP8, kind="ExternalInput").ap()
    # s_out[p, 4*(2g+c)+m] = sum_j exp(logits[row 128m+p, j]) over bank (g,c)
    s_out = nc.dram_tensor("s_out", [P, 32], F32, kind="ExternalOutput").ap()

    with tile.TileContext(nc) as tc:
        with (
            tc.tile_pool(name="persist", bufs=1) as persist,
            tc.tile_pool(name="psp", bufs=8, space="PSUM") as psp,
        ):
            _body(nc, persist, psp, qPK, cPK, bPK, s_out)

    nc.compile()
    return nc


def _body(nc, persist, psp, qPK, cPK, bPK, s_out):
    # ---- DMA front, strict consumption order on the SP ring.
    qms = []
    for m in range(MT):
        qm = persist.tile([P, 2 * KT2, P], FP8, tag=f"qm_{m}", name=f"qm_{m}")
        nc.sync.dma_start(out=qm, in_=qPK[m])
        qms.append(qm)
    g0chunks = []
    for ct in range(8):
        ch = persist.tile([P, 2, NW], FP8, tag=f"c0_{ct}", name=f"c0_{ct}")
        nc.sync.dma_start(out=ch, in_=cPK[ct])
        g0chunks.append(ch)
    bts = []
    for gc in range(6):
        bt = persist.tile([P, 2 * KT2, NW], FP8, tag=f"b_{gc}", name=f"b_{gc}")
        nc.sync.dma_start(out=bt, in_=bPK[gc])
        bts.append(bt)

    # ---- warmup: dummy Exp triggers the one-time activation table load;
    # dummy DoubleRow matmuls (N=256, ~230ns cold) keep the PE busy through
    # the HAM window so real matmuls run at 2.4 GHz once their data lands.
    scr8 = persist.tile([P, 2, 256], FP8, tag="scr8")
    nc.gpsimd.memset(scr8, 0.0)
    scrf = persist.tile([P, 1], F32, tag="scrf")
    nc.gpsimd.memset(scrf, 0.0)
    dumo = persist.tile([P, 1], F32, tag="dumo")
    nc.scalar.activation(dumo, scrf, AF.Exp)

    bkw = psp.tile([P, NW], F32, tag="bk", name="bk_warm", bufs=8)
    for w in range(N_WARM):
        nc.tensor.matmul(
            bkw[:, 0:256],
            lhsT=scr8[:, :, 0:P],
            rhs=scr8,
            start=True,
            stop=True,
            perf_mode=DR,
        )

    # ---- main loop: 32 banks of [128 rows x 512 cols]. m innermost so each
    # ans chunk feeds 4 matmuls; bank (g,c,m) accumulates over t and is
    # drained by ScalarE in-place Exp + VectorE row-sum as soon as its last
    # matmul retires (its next reuse is ~3.5us later; no PE stall).
    s_sb_a = persist.tile([P, 24], F32, tag="s_sb_a")
    s_sb_b = persist.tile([P, 8], F32, tag="s_sb_b")
    for g in range(G):
        for c in range(2):
            banks = [
                psp.tile([P, NW], F32, tag="bk", name=f"bk_{g}_{c}_{m}", bufs=8)
                for m in range(MT)
            ]
            for t in range(KT2):
                if g == 0:
                    rhs = g0chunks[c * KT2 + t]
                else:
                    rhs = bts[(g - 1) * 2 + c][:, 2 * t : 2 * t + 2, :]
                for m in range(MT):
                    nc.tensor.matmul(
                        banks[m],
                        lhsT=qms[m][:, 2 * t : 2 * t + 2, :],
                        rhs=rhs,
                        start=(t == 0),
                        stop=(t == KT2 - 1),
                        perf_mode=DR,
                    )
                    if t == KT2 - 1:
                        col = (g * 2 + c) * MT + m
                        acc = (
                            s_sb_a[:, col : col + 1]
                            if col < 24
                            else s_sb_b[:, col - 24 : col - 23]
                        )
                        nc.scalar.activation(
                            banks[m], banks[m], AF.Exp, scale=float(EXP_SCALE)
                        )
                        nc.vector.tensor_reduce(
                            acc, banks[m], axis=AX.X, op=ALU.add
                        )
        if g == G - 2:
            nc.sync.dma_start(out=s_out[:, 0:24], in_=s_sb_a)

    nc.sync.dma_start(out=s_out[:, 24:32], in_=s_sb_b)


_CACHE = {}


def _get_program():
    if "nc" not in _CACHE:
        _CACHE["nc"] = _build_program()
    return _CACHE["nc"]


def _make_in_maps(que, ans):
    """Normalize rows (folding the cosine norms into the quantization scale),
    quantize to fp8e4m3, and pack into the on-chip tile layouts. Also returns
    the exact host-computed diagonal logits."""
    fp8 = mybir.dt.np(FP8)
    que = np.asarray(que, dtype=np.float32)
    ans = np.asarray(ans, dtype=np.float32)

    qn = np.maximum(np.sqrt((que.astype(np.float64) ** 2).sum(1)), EPS)
    an = np.maximum(np.sqrt((ans.astype(np.float64) ** 2).sum(1)), EPS)
    q8 = (que * (SCALE / qn[:, None]).astype(np.float32)).astype(fp8)
    a8 = (ans * (SCALE / an[:, None]).astype(np.float32)).astype(fp8)

    # diag logits (exact, f64): cos(q_i, a_i) / gamma
    diag = (que.astype(np.float64) * ans.astype(np.float64)).sum(1) / (
        qn * an * GAMA
    )

    # a8 [4096, 1024] -> [g, c, j, t, i, p] views (shared by all cores)
    a6 = a8.reshape(G, 2, NW, KT2, 2, P)
    # cPK[4c+t, p, i, j] = a6[0, c, j, t, i, p]
    cPK = np.ascontiguousarray(a6[0].transpose(0, 2, 4, 3, 1)).reshape(8, P, 2, NW)
    # bPK[2(g-1)+c, p, 2t+i, j] = a6[g, c, j, t, i, p]
    bPK = np.ascontiguousarray(a6[1:].transpose(0, 1, 5, 3, 4, 2)).reshape(
        6, P, 2 * KT2, NW
    )

    in_maps = []
    for c in range(NCORES):
        qc = q8[c * NB : (c + 1) * NB]  # [512, 1024]
        # qPK[m, p, 2t+i, mm] = qc[128m+mm, 256t+128i+p]
        qPK = np.ascontiguousarray(
            qc.reshape(MT, P, KT2, 2, P).transpose(0, 4, 2, 3, 1)
        ).reshape(MT, P, 2 * KT2, P)
        in_maps.append({"qPK": qPK, "cPK": cPK, "bPK": bPK})
    return in_maps, diag


def _finish(results, diag):
    # s_out[p, 4*(2g+c)+m]: per-bank partial softmax denominators.
    denoms = []
    for r in results:
        s = np.asarray(r["s_out"]).reshape(P, 2 * G, MT).sum(axis=1)  # [p, m]
        denoms.append(s.T.reshape(-1))  # local row order m*128+p
    denom = np.concatenate(denoms)  # [B]
    lse = np.log(denom.astype(np.float64))
    loss = np.float32(np.mean(lse - diag))
    return np.array([loss], dtype=np.float32)


def kernel(que_batch, ans_batch):
    nc = _get_program()
    in_maps, diag = _make_in_maps(np.asarray(que_batch), np.asarray(ans_batch))
    res = run_bass_kernel_spmd(nc, in_maps, list(range(NCORES)))
    return _finish(res.results, diag)


if __name__ == "__main__":
    rng = np.random.default_rng(0)
    q = rng.standard_normal((B, D), dtype=np.float32)
    a = rng.standard_normal((B, D), dtype=np.float32)
    print(kernel(q, a))


# revision 17
# speedup vs baseline: 1.1189x; 1.1189x over previous
"""Trainium2 Bass kernel for nn_BatchContrastLoss (InfoNCE-style contrastive loss).

Reference computation:
    sim[i,j]  = cos(que_i, ans_j)            (eps-guarded norms)
    logits    = sim / 0.07
    loss      = -mean_i(log_softmax(logits, axis=1)[i,i])

Sharding: data-parallel over rows of que across 8 NeuronCores. Each core
computes its [512, 4096] logits slab against the full ans batch and reduces
each row to a softmax denominator sum_j exp(logits[i,j]). The host takes
log + mean and subtracts the diagonal (the "all-reduce" of the hint).

Design (v6; baseline v1 was 101us, DVE/ScalarE-bound):
  - Row norms are folded into the fp8 quantization on the host: rows are
    normalized to unit length, scaled by 16 (keeps e4m3 mantissa well fed),
    and quantized. The device then needs NO norm computation at all: psum =
    (16*qhat)·(16*ahat) = 256*cos, and the exp drain folds 1/(256*gamma)
    into its free affine scale. The diagonal logits_ii are computed exactly
    on the host in f64 (O(B*D), negligible).
  - fp8e4m3 DoubleRow matmuls (K=256/instr, N=512 moving cols): measured
    216ns issue-to-issue warm => 128 MMs ~ 27.6us/core floor. LDWEIGHTS
    overlaps in the PE's reorder window.
  - Loop (g: 1024-col group, m: 128-row tile, c: bank, t: k-pair): one
    [128,1024] 2-bank PSUM tile per (g,m), drained in-place by a single
    ScalarE Exp with fused row-sum accumulation ((1024+352)/1.2 ~ 1.15us;
    16 total => ScalarE ~60% busy, off the critical path).
  - DMA: all on the SP HWDGE ring in consumption order. Measured behavior:
    ~630ns issue per dma_start, ~1.9us pipe-fill, then ~0.45us/piece fixed +
    ~550GB/s marginal => 1MB groups for ans (407GB/s sustained), 128KB
    per-m-tile pieces for que. First matmul is gated on qm[0]+ag[0] ~13.3us.
  - The PE clock gate (HAM) needs ~3.4us of *continuous* activity aligned to
    its free-running window to unthrottle 1.2->2.4 GHz, and any idle gap
    resets it. N_WARM=28 dummy matmuls (N=256, ~230ns each) bridge the PE
    from block start (~7.3us) to the DMA gate with no gap, so real matmuls
    run warm from the first instruction. A dummy Exp pulls the one-time
    ~2.7us activation table load off the critical path.
"""

import numpy as np

import concourse.bass as bass
import concourse.mybir as mybir
import concourse.tile as tile
from concourse import bacc
from concourse.bass_utils import run_bass_kernel_spmd

# Problem constants (self-contained; the harness provides only the inputs).
B = 4096  # rows of que_batch / ans_batch
D = 1024  # feature dim
NCORES = 8
NB = B // NCORES  # local que rows per core = 512
P = 128  # SBUF partitions
KT2 = 4  # k-pair tiles (each DoubleRow matmul contracts 256 dims)
NW = 512  # matmul moving width = one fp32 PSUM bank
G = 4  # ans column groups of 1024
MT = NB // P  # 4 row tiles of 128
GAMA = 0.07
EPS = 1e-8
SCALE = 16.0  # host quantization scale on unit rows
EXP_SCALE = 1.0 / (SCALE * SCALE * GAMA)  # psum -> logits
N_WARM = 28  # dummy matmuls bridging block start -> first-data gate

F32 = mybir.dt.float32
FP8 = mybir.dt.float8e4  # e4m3
DR = mybir.MatmulPerfMode.DoubleRow
AF = mybir.ActivationFunctionType

OUTPUT_NAMES = ["s_out"]


def _build_program():
    nc = bacc.Bacc(
        "TRN2", target_bir_lowering=False, debug=False, num_devices=NCORES
    )

    # qPK[m, p, 2t+i, mm] = q16hat_fp8[local row 128m+mm, d=256t+128i+p]
    qPK = nc.dram_tensor("qPK", [MT, P, 2 * KT2, P], FP8, kind="ExternalInput").ap()
    # aPK[g, p, 2t+i, j] = a16hat_fp8[col 1024g+j, d=256t+128i+p]
    aPK = nc.dram_tensor("aPK", [G, P, 2 * KT2, 1024], FP8, kind="ExternalInput").ap()
    # s_out[p, 4g+m] = sum_{j in group g} exp(logits[row 128m+p, j])
    s_out = nc.dram_tensor("s_out", [P, G * MT], F32, kind="ExternalOutput").ap()

    with tile.TileContext(nc) as tc:
        with (
            tc.tile_pool(name="persist", bufs=1) as persist,
            tc.tile_pool(name="psp", bufs=4, space="PSUM") as psp,
        ):
            _body(nc, persist, psp, qPK, aPK, s_out)

    nc.compile()
    return nc


def _body(nc, persist, psp, qPK, aPK, s_out):
    # ---- DMA front, all on the SP HWDGE ring in consumption order.
    qms = []
    ags = []

    def dma_q(m):
        qm = persist.tile([P, 2 * KT2, P], FP8, tag=f"qm_{m}", name=f"qm_{m}")
        nc.sync.dma_start(out=qm, in_=qPK[m])
        qms.append(qm)

    def dma_a(g):
        a = persist.tile([P, 2 * KT2, 1024], FP8, tag=f"ag_{g}", name=f"ag_{g}")
        nc.sync.dma_start(out=a, in_=aPK[g])
        ags.append(a)

    dma_q(0)
    dma_a(0)
    for m in range(1, MT):
        dma_q(m)
    for g in range(1, G):
        dma_a(g)

    # ---- warmup: dummy Exp triggers the one-time activation table load;
    # dummy DoubleRow matmuls keep the PE busy with no gap from block start
    # until the qm[0]+ag[0] gate, so the HAM clock is warm for every real
    # matmul. All on zeroed scratch, off to the side.
    scr8 = persist.tile([P, 2, 256], FP8, tag="scr8")
    nc.gpsimd.memset(scr8, 0.0)
    scrf = persist.tile([P, 1], F32, tag="scrf")
    nc.gpsimd.memset(scrf, 0.0)
    dumo = persist.tile([P, 1], F32, tag="dumo")
    nc.scalar.activation(dumo, scrf, AF.Exp)

    ppw = psp.tile([P, 2 * NW], F32, tag="pp", name="pp_warm")
    for w in range(N_WARM):
        nc.tensor.matmul(
            ppw[:, 0:256],
            lhsT=scr8[:, :, 0:P],
            rhs=scr8,
            start=True,
            stop=True,
            perf_mode=DR,
        )

    # ---- main loop: 16 (g, m) slabs of [128 rows x 1024 cols], each one
    # 2-bank PSUM tile built by 8 DoubleRow matmuls, drained in-place by a
    # single Exp with fused row-sum accumulation. The first 12 accumulator
    # columns ship out early so only a tiny DMA trails the last drain.
    s_sb_a = persist.tile([P, 12], F32, tag="s_sb_a")
    s_sb_b = persist.tile([P, 4], F32, tag="s_sb_b")
    for g in range(G):
        for m in range(MT):
            pp = psp.tile([P, 2 * NW], F32, tag="pp", name=f"pp_{g}_{m}")
            for c in range(2):
                for t in range(KT2):
                    rhs = ags[g][:, 2 * t : 2 * t + 2, c * NW : (c + 1) * NW]
                    nc.tensor.matmul(
                        pp[:, c * NW : (c + 1) * NW],
                        lhsT=qms[m][:, 2 * t : 2 * t + 2, :],
                        rhs=rhs,
                        start=(t == 0),
                        stop=(t == KT2 - 1),
                        perf_mode=DR,
                    )
            col = g * MT + m
            acc = (
                s_sb_a[:, col : col + 1]
                if col < 12
                else s_sb_b[:, col - 12 : col - 11]
            )
            nc.scalar.activation(
                pp,
                pp,
                AF.Exp,
                scale=float(EXP_SCALE),
                accum_out=acc,
            )
        if g == G - 2:
            nc.sync.dma_start(out=s_out[:, 0:12], in_=s_sb_a)

    nc.sync.dma_start(out=s_out[:, 12:16], in_=s_sb_b)


_CACHE = {}


def _get_program():
    if "nc" not in _CACHE:
        _CACHE["nc"] = _build_program()
    return _CACHE["nc"]


def _make_in_maps(que, ans):
    """Normalize rows (folding the cosine norms into the quantization scale),
    quantize to fp8e4m3, and pack into the on-chip tile layouts. Also returns
    the exact host-computed diagonal logits."""
    fp8 = mybir.dt.np(FP8)
    que = np.asarray(que, dtype=np.float32)
    ans = np.asarray(ans, dtype=np.float32)

    qn = np.maximum(np.sqrt((que.astype(np.float64) ** 2).sum(1)), EPS)
    an = np.maximum(np.sqrt((ans.astype(np.float64) ** 2).sum(1)), EPS)
    q8 = (que * (SCALE / qn[:, None]).astype(np.float32)).astype(fp8)
    a8 = (ans * (SCALE / an[:, None]).astype(np.float32)).astype(fp8)

    # diag logits (exact, f64): cos(q_i, a_i) / gamma
    diag = (que.astype(np.float64) * ans.astype(np.float64)).sum(1) / (
        qn * an * GAMA
    )

    # aPK[g, p, 2t+i, j] = a8[1024g+j, 256t+128i+p]  (shared by all cores)
    aPK = np.ascontiguousarray(
        a8.reshape(G, 1024, KT2, 2, P).transpose(0, 4, 2, 3, 1)
    ).reshape(G, P, 2 * KT2, 1024)

    in_maps = []
    for c in range(NCORES):
        qc = q8[c * NB : (c + 1) * NB]  # [512, 1024]
        # qPK[m, p, 2t+i, mm] = qc[128m+mm, 256t+128i+p]
        qPK = np.ascontiguousarray(
            qc.reshape(MT, P, KT2, 2, P).transpose(0, 4, 2, 3, 1)
        ).reshape(MT, P, 2 * KT2, P)
        in_maps.append({"qPK": qPK, "aPK": aPK})
    return in_maps, diag


def _finish(results, diag):
    # s_out[p, 4g+m]: per-group partial softmax denominators.
    denoms = []
    for r in results:
        s = np.asarray(r["s_out"]).reshape(P, G, MT).sum(axis=1)  # [p, m]
        denoms.append(s.T.reshape(-1))  # local row order m*128+p
    denom = np.concatenate(denoms)  # [B]
    lse = np.log(denom.astype(np.float64))
    loss = np.float32(np.mean(lse - diag))
    return np.array([loss], dtype=np.float32)


def kernel(que_batch, ans_batch):
    nc = _get_program()
    in_maps, diag = _make_in_maps(np.asarray(que_batch), np.asarray(ans_batch))
    res = run_bass_kernel_spmd(nc, in_maps, list(range(NCORES)))
    return _finish(res.results, diag)


if __name__ == "__main__":
    rng = np.random.default_rng(0)
    q = rng.standard_normal((B, D), dtype=np.float32)
    a = rng.standard_normal((B, D), dtype=np.float32)
    print(kernel(q, a))
